# revision 14
# baseline (speedup 1.0000x reference)
"""Trainium2 Bass kernel for nn_BasicBlock_CSAFR (topk_masking).

Computation per sample n (fully sample-independent -> pure data parallel,
8 images per NeuronCore across 8 cores):

  out1   = relu(bn1(x))                                  (512,32,32)
  gsum   = out1.sum(H,W)                                 (512,)
  z      = 5*(fc1_w @ (gsum/1024) + fc1_b)               (240,)
  hp     = softplus(z);  sig = sigmoid(z)
  logits = fc2_w @ (hp/5) + fc2_b                        (10,)   -> pred_probe
  e      = onehot(top1)+onehot(top2) of logits
  dh     = fc2_w.T @ e ; dpre = dh*sig ; s = fc1_w.T @ dpre
  mask   = 2*softmax(s)                                  (512,)
  h      = conv2(relu(bn2(conv1(out1*mask))))            3x3 SAME convs, 512->512
  out    = x + h

Convs run on the TensorEngine as 9 shifted matmuls per output tile
(channels on partitions, zero-padded 34x34 images in SBUF) in fp8-e4m3
DoubleRow mode (2 MACs/cell/cycle, 256-channel contraction per matmul)
with fp32 PSUM accumulation.  Activations/weights are pre-scaled
(SA=1024, SW=64) to sit in the fp8 normal range; scales fold into the
bn eviction and the residual add.  The final out = x + h is dominated
by exact fp32 x, so fp8 conv error (~1e-3 absmax) is far inside any
reasonable tolerance.  The probe/mask chain runs in fp32 using small
matmuls (incl. matmul-based transposes / partition broadcasts /
partition reductions).  softplus/sigmoid are built from the Exp LUT +
a deg-8 log1p polynomial on DVE (no Softplus table exists on trn2).
"""

import os
import sys

for _p in ("/opt/trn_rl_repo", "/root/.axon_site/_ro/trn_rl_repo"):
    if os.path.isdir(_p) and _p not in sys.path:
        sys.path.append(_p)

import numpy as np
import ml_dtypes

import concourse.bass as bass
import concourse.tile as tile
from concourse import bacc, mybir
from concourse.bass_utils import run_bass_kernel_spmd

F32 = mybir.dt.float32
BF16 = mybir.dt.bfloat16
FP8 = mybir.dt.float8e4
AF = mybir.ActivationFunctionType
OP = mybir.AluOpType
AX = mybir.AxisListType
DR = mybir.MatmulPerfMode.DoubleRow

N, C, H, W = 64, 512, 32, 32
HID, NCLS = 240, 10
NCORES = 8
NIMG = N // NCORES  # images per core
NQ = C // 128       # channel chunks
HB = HID - 128      # second fc-hidden chunk (112)
SW = 64.0           # fp8 weight scale
SA = 1024.0         # fp8 activation scale
NBUF = 4            # image pipeline depth

# log1p(u) ~= sum_k c_k u^k on [0,1]; max abs err 1.4e-7
LOG1P_C = [0.9999998102, -0.4999744938, 0.3327617657, -0.2449961172,
           0.1775702399, -0.1078536792, 0.04421419234, -0.008574676205]

_cache = {}


def build_program(n_img=NIMG):
    """Build + compile the per-core Bass program (n_img images)."""
    nc = bacc.Bacc("TRN2", target_bir_lowering=False, debug=False)

    xd = nc.dram_tensor("x", [n_img, C, H, W], F32, kind="ExternalInput")
    w1d = nc.dram_tensor("w1t", [9, 128, 2, 2, C], FP8, kind="ExternalInput")
    w2d = nc.dram_tensor("w2t", [9, 128, 2, 2, C], FP8, kind="ExternalInput")
    bnd = nc.dram_tensor("bnp", [C, 4], F32, kind="ExternalInput")
    f1td = nc.dram_tensor("fc1wt", [128, NQ, HID], F32, kind="ExternalInput")
    f1ad = nc.dram_tensor("fc1wa", [128, C], F32, kind="ExternalInput")
    f1bd = nc.dram_tensor("fc1wb", [HB, C], F32, kind="ExternalInput")
    f2tad = nc.dram_tensor("fc2wt5a", [128, NCLS], F32, kind="ExternalInput")
    f2tbd = nc.dram_tensor("fc2wt5b", [HB, NCLS], F32, kind="ExternalInput")
    f2d = nc.dram_tensor("fc2w", [NCLS, HID], F32, kind="ExternalInput")
    b1ad = nc.dram_tensor("fc1b5a", [128, 1], F32, kind="ExternalInput")
    b1bd = nc.dram_tensor("fc1b5b", [HB, 1], F32, kind="ExternalInput")
    b2d = nc.dram_tensor("fc2brow", [1, NCLS], F32, kind="ExternalInput")

    od = nc.dram_tensor("out", [n_img, C, H, W], F32, kind="ExternalOutput")
    pd = nc.dram_tensor("probe", [1, n_img * NCLS], F32, kind="ExternalOutput")

    xa, oa = xd.ap(), od.ap()

    with tile.TileContext(nc) as tc, tc.tile_pool(name="pp", bufs=1) as pp:
        def ptile(shape, dtype, name):
            return pp.tile(shape, dtype, name=name, tag=name)

        # ---- persistent tiles -------------------------------------------
        w1t = ptile([128, 9, 2, 2, C], FP8, "w1t_s")
        w2t = ptile([128, 9, 2, 2, C], FP8, "w2t_s")
        for tap in range(9):
            nc.sync.dma_start(w1t[:, tap, :, :, :], w1d.ap()[tap])
            nc.sync.dma_start(w2t[:, tap, :, :, :], w2d.ap()[tap])

        sbn = []
        for q in range(NQ):
            t = ptile([128, 4], F32, f"sbn{q}")
            nc.sync.dma_start(t[:], bnd.ap()[128 * q:128 * (q + 1), :])
            sbn.append(t)

        fc1wt = ptile([128, NQ, HID], F32, "fc1wt_s")
        nc.sync.dma_start(fc1wt[:], f1td.ap()[:])
        fc1wa = ptile([128, C], F32, "fc1wa_s")
        nc.sync.dma_start(fc1wa[:], f1ad.ap()[:])
        fc1wb_f = ptile([128, C], F32, "fc1wb_s")
        fc1wb = fc1wb_f[0:HB, :]
        nc.sync.dma_start(fc1wb, f1bd.ap()[:])
        fc2wt5a = ptile([128, NCLS], F32, "fc2wt5a_s")
        nc.sync.dma_start(fc2wt5a[:], f2tad.ap()[:])
        fc2wt5b_f = ptile([128, NCLS], F32, "fc2wt5b_s")
        fc2wt5b = fc2wt5b_f[0:HB, :]
        nc.sync.dma_start(fc2wt5b, f2tbd.ap()[:])
        fc2w_f = ptile([128, HID], F32, "fc2w_s")
        fc2w = fc2w_f[0:NCLS, :]
        nc.sync.dma_start(fc2w, f2d.ap()[:])
        fc1b5a = ptile([128, 1], F32, "fc1b5a_s")
        nc.sync.dma_start(fc1b5a[:], b1ad.ap()[:])
        fc1b5b_f = ptile([128, 1], F32, "fc1b5b_s")
        fc1b5b = fc1b5b_f[0:HB, :]
        nc.sync.dma_start(fc1b5b, b1bd.ap()[:])
        fc2brow_f = ptile([128, NCLS], F32, "fc2brow_s")
        fc2brow = fc2brow_f[0:1, :]
        nc.sync.dma_start(fc2brow, b2d.ap()[:])

        ones_row_f = ptile([128, 128], F32, "ones_row")
        ones_row = ones_row_f[0:1, :]
        nc.vector.memset(ones_row, 1.0)
        ones_col = ptile([128, 1], F32, "ones_col")
        nc.vector.memset(ones_col[:], 1.0)

        probe_acc_f = ptile([128, n_img * NCLS], F32, "probe_acc")
        probe_acc = probe_acc_f[0:1, :]

        # activation tiles (rotating over images); fp8 pads stay 0
        o1s = [ptile([128, NQ, H, W], BF16, f"o1_{i}") for i in range(NBUF)]
        mts = [ptile([128, NQ, 34, 34], FP8, f"mt{i}") for i in range(NBUF)]
        hts = [ptile([128, NQ, 34, 34], FP8, f"ht{i}") for i in range(NBUF)]
        for t in mts + hts:
            nc.vector.memset(t[:], 0.0)

        with (
            tc.tile_pool(name="xp", bufs=6) as xp,
            tc.tile_pool(name="xr", bufs=4) as xr,
            tc.tile_pool(name="ost", bufs=4) as ost,
            tc.tile_pool(name="gsp", bufs=2) as gsp,
            tc.tile_pool(name="exp_p", bufs=2) as exp_p,
            tc.tile_pool(name="mcp", bufs=2) as mcp,
            tc.tile_pool(name="smp", bufs=2) as smp,
            tc.tile_pool(name="cps", bufs=3, space="PSUM") as cps,
            tc.tile_pool(name="pps", bufs=4, space="PSUM") as pps,
        ):
            def stile(shape, name):
                return smp.tile(shape, F32, name=name, tag=name)

            def softplus_sigmoid(pre_ps, bias_ap, P, sfx):
                """hp = softplus(5*pre+bias5), sig = sigmoid(same), on P rows."""
                z = stile([P, 1], f"z_{sfx}")
                nc.vector.tensor_scalar(z[:], pre_ps[:], 5.0, bias_ap, OP.mult, OP.add)
                nz = stile([P, 1], f"nz_{sfx}")
                nc.vector.tensor_scalar(nz[:], z[:], -1.0, None, OP.mult)
                mz = stile([P, 1], f"mz_{sfx}")
                nc.vector.tensor_tensor(mz[:], z[:], nz[:], op=OP.min)
                u = stile([P, 1], f"u_{sfx}")
                nc.scalar.activation(u[:], mz[:], AF.Exp)
                q = stile([P, 1], f"q_{sfx}")
                nc.vector.tensor_scalar(q[:], u[:], LOG1P_C[-1], None, OP.mult)
                for ck in reversed(LOG1P_C[:-1]):
                    nc.vector.scalar_tensor_tensor(q[:], q[:], float(ck), u[:],
                                                   op0=OP.add, op1=OP.mult)
                rz = stile([P, 1], f"rz_{sfx}")
                nc.vector.tensor_scalar(rz[:], z[:], 0.0, None, OP.max)
                hp = stile([P, 1], f"hp_{sfx}")
                nc.vector.tensor_add(hp[:], rz[:], q[:])
                # sigmoid: r = 1/(1+u); sig = (1-r) + m*(2r-1), m = [z>=0]
                m = stile([P, 1], f"m_{sfx}")
                nc.vector.tensor_scalar(m[:], z[:], 0.0, None, OP.is_ge)
                t1 = stile([P, 1], f"t1_{sfx}")
                nc.vector.tensor_scalar(t1[:], u[:], 1.0, None, OP.add)
                r = stile([P, 1], f"r_{sfx}")
                nc.vector.reciprocal(r[:], t1[:])
                t2 = stile([P, 1], f"t2_{sfx}")
                nc.vector.tensor_scalar(t2[:], r[:], 2.0, -1.0, OP.mult, OP.add)
                t3 = stile([P, 1], f"t3_{sfx}")
                nc.vector.tensor_scalar(t3[:], r[:], -1.0, 1.0, OP.mult, OP.add)
                t4 = stile([P, 1], f"t4_{sfx}")
                nc.vector.tensor_mul(t4[:], m[:], t2[:])
                sg = stile([P, 1], f"sg_{sfx}")
                nc.vector.tensor_add(sg[:], t4[:], t3[:])
                return hp, sg

            def stage_a(n):
                """preproc + probe chain + mask multiply for image n."""
                o1, mt = o1s[n % NBUF], mts[n % NBUF]
                gs = gsp.tile([128, NQ], F32, name="gs")
                for q in range(NQ):
                    xin = xp.tile([128, H, W], F32, name="xin")
                    nc.sync.dma_start(xin[:], xa[n, 128 * q:128 * (q + 1), :, :])
                    nc.scalar.activation(
                        o1[:, q], xin[:], AF.Relu,
                        bias=sbn[q][:, 1:2], scale=sbn[q][:, 0:1],
                        accum_out=gs[:, q:q + 1],
                    )

                pre1a = pps.tile([128, 1], F32, name="pre1a", tag="pps")
                for q in range(NQ):
                    nc.tensor.matmul(pre1a[:], fc1wt[:, q, 0:128], gs[:, q:q + 1],
                                     start=(q == 0), stop=(q == NQ - 1))
                pre1b = pps.tile([HB, 1], F32, name="pre1b", tag="pps")
                for q in range(NQ):
                    nc.tensor.matmul(pre1b[:], fc1wt[:, q, 128:HID], gs[:, q:q + 1],
                                     start=(q == 0), stop=(q == NQ - 1))

                hp_a, sg_a = softplus_sigmoid(pre1a, fc1b5a[:, :], 128, "a")
                hp_b, sg_b = softplus_sigmoid(pre1b, fc1b5b, HB, "b")

                lg = pps.tile([1, NCLS], F32, name="lg", tag="pps")
                nc.tensor.matmul(lg[:], hp_a[:], fc2wt5a[:], start=True, stop=False)
                nc.tensor.matmul(lg[:], hp_b[:], fc2wt5b, start=False, stop=True)

                pslice = probe_acc[:, NCLS * n:NCLS * (n + 1)]
                nc.vector.tensor_add(pslice, lg[:], fc2brow)

                m1 = stile([1, 1], "m1")
                nc.vector.reduce_max(m1[:], pslice, axis=AX.X)
                eq1 = stile([1, NCLS], "eq1")
                nc.vector.tensor_scalar(eq1[:], pslice, m1[:], None, OP.is_equal)
                tmp = stile([1, NCLS], "tmp")
                nc.vector.scalar_tensor_tensor(tmp[:], eq1[:], -1e30, pslice,
                                               op0=OP.mult, op1=OP.add)
                m2 = stile([1, 1], "m2")
                nc.vector.reduce_max(m2[:], tmp[:], axis=AX.X)
                eq2 = stile([1, NCLS], "eq2")
                nc.vector.tensor_scalar(eq2[:], tmp[:], m2[:], None, OP.is_equal)
                e_row = stile([1, NCLS], "e_row")
                nc.vector.tensor_add(e_row[:], eq1[:], eq2[:])

                eT = pps.tile([NCLS, 1], F32, name="eT", tag="pps")
                nc.tensor.matmul(eT[:], e_row[:], ones_row[:, 0:1], start=True, stop=True)
                eT_sb = stile([NCLS, 1], "eT_sb")
                nc.vector.tensor_copy(eT_sb[:], eT[:])

                dh_a = pps.tile([128, 1], F32, name="dh_a", tag="pps")
                nc.tensor.matmul(dh_a[:], fc2w[:, 0:128], eT_sb[:], start=True, stop=True)
                dh_b = pps.tile([HB, 1], F32, name="dh_b", tag="pps")
                nc.tensor.matmul(dh_b[:], fc2w[:, 128:HID], eT_sb[:], start=True, stop=True)

                dpre_a = stile([128, 1], "dpre_a")
                nc.vector.tensor_mul(dpre_a[:], dh_a[:], sg_a[:])
                dpre_b = stile([HB, 1], "dpre_b")
                nc.vector.tensor_mul(dpre_b[:], dh_b[:], sg_b[:])

                exps = exp_p.tile([128, NQ], F32, name="exps")
                for q in range(NQ):
                    s_ps = pps.tile([128, 1], F32, name="s_ps", tag="pps")
                    nc.tensor.matmul(s_ps[:], fc1wa[:, 128 * q:128 * (q + 1)],
                                     dpre_a[:], start=True, stop=False)
                    nc.tensor.matmul(s_ps[:], fc1wb[:, 128 * q:128 * (q + 1)],
                                     dpre_b[:], start=False, stop=True)
                    nc.scalar.activation(exps[:, q:q + 1], s_ps[:], AF.Exp)

                zp = pps.tile([1, 1], F32, name="zp", tag="pps")
                for q in range(NQ):
                    nc.tensor.matmul(zp[:], exps[:, q:q + 1], ones_col[:],
                                     start=(q == 0), stop=(q == NQ - 1))
                r_sb = stile([1, 1], "r_sb")
                nc.vector.reciprocal(r_sb[:], zp[:])
                bc = pps.tile([128, 1], F32, name="bc", tag="pps")
                nc.tensor.matmul(bc[:], ones_row, r_sb[:], start=True, stop=True)

                # mask columns scaled by SA (fp8 activation scale)
                mcol = mcp.tile([128, NQ], F32, name="mcol")
                for q in range(NQ):
                    nc.vector.scalar_tensor_tensor(mcol[:, q:q + 1], exps[:, q:q + 1],
                                                   2.0 * SA, bc[:],
                                                   op0=OP.mult, op1=OP.mult)
                # masked_fp8 = out1_bf16 * (mask*SA) into padded interior
                for q in range(NQ):
                    nc.vector.tensor_scalar(mt[:, q, 1:33, 1:33], o1[:, q],
                                            mcol[:, q:q + 1], None, OP.mult)

            def stage_b1(n):
                """conv1 + bn2/relu for image n (fp8 DR)."""
                mt, ht = mts[n % NBUF], hts[n % NBUF]
                for co in range(NQ):
                    for hh in range(2):
                        ps = cps.tile([128, 16, 32], F32, name="ps1", tag="cps")
                        k = 0
                        for tap in range(9):
                            dy, dx = tap // 3, tap % 3
                            for pr in range(2):
                                nc.tensor.matmul(
                                    ps[:],
                                    w1t[:, tap, pr, :, 128 * co:128 * (co + 1)],
                                    mt[:, 2 * pr:2 * pr + 2,
                                       dy + 16 * hh:dy + 16 * hh + 16, dx:dx + 32],
                                    start=(k == 0), stop=(k == 17), perf_mode=DR,
                                )
                                k += 1
                        nc.scalar.activation(
                            ht[:, co, 1 + 16 * hh:17 + 16 * hh, 1:33], ps[:],
                            AF.Relu, bias=sbn[co][:, 3:4], scale=sbn[co][:, 2:3],
                        )

            def stage_b2(n):
                """conv2 + residual + store for image n (fp8 DR)."""
                ht = hts[n % NBUF]
                for co in range(NQ):
                    for hh in range(2):
                        ps = cps.tile([128, 16, 32], F32, name="ps2", tag="cps")
                        k = 0
                        for tap in range(9):
                            dy, dx = tap // 3, tap % 3
                            for pr in range(2):
                                nc.tensor.matmul(
                                    ps[:],
                                    w2t[:, tap, pr, :, 128 * co:128 * (co + 1)],
                                    ht[:, 2 * pr:2 * pr + 2,
                                       dy + 16 * hh:dy + 16 * hh + 16, dx:dx + 32],
                                    start=(k == 0), stop=(k == 17), perf_mode=DR,
                                )
                                k += 1
                        xres = xr.tile([128, 16, 32], F32, name="xres")
                        nc.sync.dma_start(
                            xres[:],
                            xa[n, 128 * co:128 * (co + 1), 16 * hh:16 * hh + 16, :])
                        o_sb = ost.tile([128, 16, 32], F32, name="o_sb")
                        nc.vector.scalar_tensor_tensor(o_sb[:], ps[:], 1.0 / (SW * SA),
                                                       xres[:], op0=OP.mult, op1=OP.add)
                        nc.sync.dma_start(
                            oa[n, 128 * co:128 * (co + 1), 16 * hh:16 * hh + 16, :],
                            o_sb[:])

            # software pipeline, conv2 staggered one stage behind conv1 so
            # every stage's dependencies are long since complete:
            #   a(0) a(1) b1(0) [a(2) b1(1) b2(0)] [a(3) b1(2) b2(1)] ...
            stage_a(0)
            if n_img > 1:
                stage_a(1)
            stage_b1(0)
            for n in range(n_img):
                if n + 2 < n_img:
                    stage_a(n + 2)
                if n + 1 < n_img:
                    stage_b1(n + 1)
                stage_b2(n)

            nc.sync.dma_start(pd.ap()[:], probe_acc)

    nc.compile()
    return nc


def prep_inputs(x, bn1_g, bn1_b, bn1_m, bn1_v, conv1_w,
                bn2_g, bn2_b, bn2_m, bn2_v, conv2_w,
                fc1_w, fc1_b, fc2_w, fc2_b, n_img=NIMG):
    """Host-side parameter prep -> list of per-core input maps."""
    eps = np.float32(1e-5)
    s1 = (bn1_g / np.sqrt(bn1_v + eps)).astype(np.float32)
    b1 = (bn1_b - bn1_m * s1).astype(np.float32)
    s2 = (bn2_g / np.sqrt(bn2_v + eps)).astype(np.float32)
    b2 = (bn2_b - bn2_m * s2).astype(np.float32)

    def wprep(w):
        # (Cout, Cin, 3, 3)*SW -> [tap, ci_in, pair, slab, co] fp8 e4m3
        wt = (w * SW).transpose(2, 3, 1, 0).reshape(9, 2, 2, 128, C)
        wt = wt.transpose(0, 3, 1, 2, 4)  # (9, 128, 2, 2, C)
        return np.ascontiguousarray(wt).astype(ml_dtypes.float8_e4m3)

    common = {
        "w1t": wprep(conv1_w),
        "w2t": wprep(conv2_w),
        # columns: s1, b1, s2/SW (psum scale), SA*b2 (fp8 out scale)
        "bnp": np.ascontiguousarray(
            np.stack([s1, b1, s2 / SW, SA * b2], axis=1)).astype(np.float32),
        "fc1wt": np.ascontiguousarray(
            (fc1_w.T / 1024.0).astype(np.float32)
            .reshape(NQ, 128, HID).transpose(1, 0, 2)),
        "fc1wa": np.ascontiguousarray(fc1_w[0:128]).astype(np.float32),
        "fc1wb": np.ascontiguousarray(fc1_w[128:HID]).astype(np.float32),
        "fc2wt5a": np.ascontiguousarray((fc2_w.T / 5.0)[0:128]).astype(np.float32),
        "fc2wt5b": np.ascontiguousarray((fc2_w.T / 5.0)[128:HID]).astype(np.float32),
        "fc2w": np.ascontiguousarray(fc2_w).astype(np.float32),
        "fc1b5a": (5.0 * fc1_b)[0:128].reshape(128, 1).astype(np.float32),
        "fc1b5b": (5.0 * fc1_b)[128:HID].reshape(HB, 1).astype(np.float32),
        "fc2brow": fc2_b.reshape(1, NCLS).astype(np.float32),
    }
    n_cores = x.shape[0] // n_img
    maps = []
    for i in range(n_cores):
        m = dict(common)
        m["x"] = np.ascontiguousarray(x[i * n_img:(i + 1) * n_img]).astype(np.float32)
        maps.append(m)
    return maps


def kernel(**inputs):
    inputs = {k: np.asarray(v) for k, v in inputs.items()}
    if "nc" not in _cache:
        _cache["nc"] = build_program()
    nc = _cache["nc"]
    in_maps = prep_inputs(**inputs)
    trace = bool(int(os.environ.get("KBENCH_TRACE", "0")))
    res = run_bass_kernel_spmd(nc, in_maps, list(range(NCORES)), trace=trace)
    _cache["last_results"] = res
    out = np.concatenate([res.results[i]["out"] for i in range(NCORES)], axis=0)
    probe = np.concatenate(
        [res.results[i]["probe"].reshape(NIMG, NCLS) for i in range(NCORES)], axis=0)
    return (out, probe)


# revision 15
# speedup vs baseline: 1.0063x; 1.0063x over previous
"""Trainium2 Bass kernel for nn_BasicBlock_CSAFR (topk_masking).

Computation per sample n (fully sample-independent -> pure data parallel,
8 images per NeuronCore across 8 cores):

  out1   = relu(bn1(x))                                  (512,32,32)
  gsum   = out1.sum(H,W)                                 (512,)
  z      = 5*(fc1_w @ (gsum/1024) + fc1_b)               (240,)
  hp     = softplus(z);  sig = sigmoid(z)
  logits = fc2_w @ (hp/5) + fc2_b                        (10,)   -> pred_probe
  e      = onehot(top1)+onehot(top2) of logits
  dh     = fc2_w.T @ e ; dpre = dh*sig ; s = fc1_w.T @ dpre
  mask   = 2*softmax(s)                                  (512,)
  h      = conv2(relu(bn2(conv1(out1*mask))))            3x3 SAME convs, 512->512
  out    = x + h

Convs run on the TensorEngine as 9 shifted matmuls per output tile
(channels on partitions, zero-padded 34x34 images in SBUF) in fp8-e4m3
DoubleRow mode (2 MACs/cell/cycle, 256-channel contraction per matmul)
with fp32 PSUM accumulation.  Activations/weights are pre-scaled
(SA=1024, SW=64) to sit in the fp8 normal range; scales fold into the
bn eviction and the residual add.  The final out = x + h is dominated
by exact fp32 x, so fp8 conv error (~1e-3 absmax) is far inside any
reasonable tolerance.  The probe/mask chain runs in fp32 using small
matmuls (incl. matmul-based transposes / partition broadcasts /
partition reductions).  softplus/sigmoid are built from the Exp LUT +
a deg-8 log1p polynomial on DVE (no Softplus table exists on trn2).
"""

import os
import sys

for _p in ("/opt/trn_rl_repo", "/root/.axon_site/_ro/trn_rl_repo"):
    if os.path.isdir(_p) and _p not in sys.path:
        sys.path.append(_p)

import numpy as np
import ml_dtypes

import concourse.bass as bass
import concourse.tile as tile
from concourse import bacc, mybir
from concourse.bass_utils import run_bass_kernel_spmd

F32 = mybir.dt.float32
BF16 = mybir.dt.bfloat16
FP8 = mybir.dt.float8e4
AF = mybir.ActivationFunctionType
OP = mybir.AluOpType
AX = mybir.AxisListType
DR = mybir.MatmulPerfMode.DoubleRow

N, C, H, W = 64, 512, 32, 32
HID, NCLS = 240, 10
NCORES = 8
NIMG = N // NCORES  # images per core
NQ = C // 128       # channel chunks
HB = HID - 128      # second fc-hidden chunk (112)
SW = 64.0           # fp8 weight scale
SA = 1024.0         # fp8 activation scale
NBUF = 3            # image pipeline depth

# log1p(u) ~= sum_k c_k u^k on [0,1]; max abs err 1.4e-7
LOG1P_C = [0.9999998102, -0.4999744938, 0.3327617657, -0.2449961172,
           0.1775702399, -0.1078536792, 0.04421419234, -0.008574676205]

_cache = {}


def build_program(n_img=NIMG):
    """Build + compile the per-core Bass program (n_img images)."""
    nc = bacc.Bacc("TRN2", target_bir_lowering=False, debug=False)

    xd = nc.dram_tensor("x", [n_img, C, H, W], F32, kind="ExternalInput")
    w1d = nc.dram_tensor("w1t", [9, 128, 2, 2, C], FP8, kind="ExternalInput")
    w2d = nc.dram_tensor("w2t", [9, 128, 2, 2, C], FP8, kind="ExternalInput")
    bnd = nc.dram_tensor("bnp", [C, 4], F32, kind="ExternalInput")
    f1td = nc.dram_tensor("fc1wt", [128, NQ, HID], F32, kind="ExternalInput")
    f1ad = nc.dram_tensor("fc1wa", [128, C], F32, kind="ExternalInput")
    f1bd = nc.dram_tensor("fc1wb", [HB, C], F32, kind="ExternalInput")
    f2tad = nc.dram_tensor("fc2wt5a", [128, NCLS], F32, kind="ExternalInput")
    f2tbd = nc.dram_tensor("fc2wt5b", [HB, NCLS], F32, kind="ExternalInput")
    f2d = nc.dram_tensor("fc2w", [NCLS, HID], F32, kind="ExternalInput")
    b1ad = nc.dram_tensor("fc1b5a", [128, 1], F32, kind="ExternalInput")
    b1bd = nc.dram_tensor("fc1b5b", [HB, 1], F32, kind="ExternalInput")
    b2d = nc.dram_tensor("fc2brow", [1, NCLS], F32, kind="ExternalInput")

    od = nc.dram_tensor("out", [n_img, C, H, W], F32, kind="ExternalOutput")
    pd = nc.dram_tensor("probe", [1, n_img * NCLS], F32, kind="ExternalOutput")

    xa, oa = xd.ap(), od.ap()

    with tile.TileContext(nc) as tc, tc.tile_pool(name="pp", bufs=1) as pp:
        def ptile(shape, dtype, name):
            return pp.tile(shape, dtype, name=name, tag=name)

        # ---- persistent tiles -------------------------------------------
        w1t = ptile([128, 9, 2, 2, C], FP8, "w1t_s")
        w2t = ptile([128, 9, 2, 2, C], FP8, "w2t_s")
        for tap in range(9):
            nc.sync.dma_start(w1t[:, tap, :, :, :], w1d.ap()[tap])
            nc.sync.dma_start(w2t[:, tap, :, :, :], w2d.ap()[tap])

        sbn = []
        for q in range(NQ):
            t = ptile([128, 4], F32, f"sbn{q}")
            nc.sync.dma_start(t[:], bnd.ap()[128 * q:128 * (q + 1), :])
            sbn.append(t)

        fc1wt = ptile([128, NQ, HID], F32, "fc1wt_s")
        nc.sync.dma_start(fc1wt[:], f1td.ap()[:])
        fc1wa = ptile([128, C], F32, "fc1wa_s")
        nc.sync.dma_start(fc1wa[:], f1ad.ap()[:])
        fc1wb_f = ptile([128, C], F32, "fc1wb_s")
        fc1wb = fc1wb_f[0:HB, :]
        nc.sync.dma_start(fc1wb, f1bd.ap()[:])
        fc2wt5a = ptile([128, NCLS], F32, "fc2wt5a_s")
        nc.sync.dma_start(fc2wt5a[:], f2tad.ap()[:])
        fc2wt5b_f = ptile([128, NCLS], F32, "fc2wt5b_s")
        fc2wt5b = fc2wt5b_f[0:HB, :]
        nc.sync.dma_start(fc2wt5b, f2tbd.ap()[:])
        fc2w_f = ptile([128, HID], F32, "fc2w_s")
        fc2w = fc2w_f[0:NCLS, :]
        nc.sync.dma_start(fc2w, f2d.ap()[:])
        fc1b5a = ptile([128, 1], F32, "fc1b5a_s")
        nc.sync.dma_start(fc1b5a[:], b1ad.ap()[:])
        fc1b5b_f = ptile([128, 1], F32, "fc1b5b_s")
        fc1b5b = fc1b5b_f[0:HB, :]
        nc.sync.dma_start(fc1b5b, b1bd.ap()[:])
        fc2brow_f = ptile([128, NCLS], F32, "fc2brow_s")
        fc2brow = fc2brow_f[0:1, :]
        nc.sync.dma_start(fc2brow, b2d.ap()[:])

        ones_row_f = ptile([128, 128], F32, "ones_row")
        ones_row = ones_row_f[0:1, :]
        nc.vector.memset(ones_row, 1.0)
        ones_col = ptile([128, 1], F32, "ones_col")
        nc.vector.memset(ones_col[:], 1.0)

        probe_acc_f = ptile([128, n_img * NCLS], F32, "probe_acc")
        probe_acc = probe_acc_f[0:1, :]

        # activation tiles (rotating over images); fp8 pads stay 0
        o1s = [ptile([128, NQ, H, W], BF16, f"o1_{i}") for i in range(NBUF)]
        mts = [ptile([128, NQ, 34, 34], FP8, f"mt{i}") for i in range(NBUF)]
        hts = [ptile([128, NQ, 34, 34], FP8, f"ht{i}") for i in range(NBUF)]
        for t in mts + hts:
            nc.vector.memset(t[:], 0.0)

        with (
            tc.tile_pool(name="xp", bufs=6) as xp,
            tc.tile_pool(name="xr", bufs=4) as xr,
            tc.tile_pool(name="ost", bufs=4) as ost,
            tc.tile_pool(name="gsp", bufs=2) as gsp,
            tc.tile_pool(name="exp_p", bufs=2) as exp_p,
            tc.tile_pool(name="mcp", bufs=2) as mcp,
            tc.tile_pool(name="smp", bufs=2) as smp,
            tc.tile_pool(name="cps", bufs=3, space="PSUM") as cps,
            tc.tile_pool(name="pps", bufs=4, space="PSUM") as pps,
        ):
            def stile(shape, name):
                return smp.tile(shape, F32, name=name, tag=name)

            def softplus_sigmoid(pre_ps, bias_ap, P, sfx):
                """hp = softplus(5*pre+bias5), sig = sigmoid(same), on P rows."""
                z = stile([P, 1], f"z_{sfx}")
                nc.vector.tensor_scalar(z[:], pre_ps[:], 5.0, bias_ap, OP.mult, OP.add)
                nz = stile([P, 1], f"nz_{sfx}")
                nc.vector.tensor_scalar(nz[:], z[:], -1.0, None, OP.mult)
                mz = stile([P, 1], f"mz_{sfx}")
                nc.vector.tensor_tensor(mz[:], z[:], nz[:], op=OP.min)
                u = stile([P, 1], f"u_{sfx}")
                nc.scalar.activation(u[:], mz[:], AF.Exp)
                q = stile([P, 1], f"q_{sfx}")
                nc.vector.tensor_scalar(q[:], u[:], LOG1P_C[-1], None, OP.mult)
                for ck in reversed(LOG1P_C[:-1]):
                    nc.vector.scalar_tensor_tensor(q[:], q[:], float(ck), u[:],
                                                   op0=OP.add, op1=OP.mult)
                rz = stile([P, 1], f"rz_{sfx}")
                nc.vector.tensor_scalar(rz[:], z[:], 0.0, None, OP.max)
                hp = stile([P, 1], f"hp_{sfx}")
                nc.vector.tensor_add(hp[:], rz[:], q[:])
                # sigmoid: r = 1/(1+u); sig = (1-r) + m*(2r-1), m = [z>=0]
                m = stile([P, 1], f"m_{sfx}")
                nc.vector.tensor_scalar(m[:], z[:], 0.0, None, OP.is_ge)
                t1 = stile([P, 1], f"t1_{sfx}")
                nc.vector.tensor_scalar(t1[:], u[:], 1.0, None, OP.add)
                r = stile([P, 1], f"r_{sfx}")
                nc.vector.reciprocal(r[:], t1[:])
                t2 = stile([P, 1], f"t2_{sfx}")
                nc.vector.tensor_scalar(t2[:], r[:], 2.0, -1.0, OP.mult, OP.add)
                t3 = stile([P, 1], f"t3_{sfx}")
                nc.vector.tensor_scalar(t3[:], r[:], -1.0, 1.0, OP.mult, OP.add)
                t4 = stile([P, 1], f"t4_{sfx}")
                nc.vector.tensor_mul(t4[:], m[:], t2[:])
                sg = stile([P, 1], f"sg_{sfx}")
                nc.vector.tensor_add(sg[:], t4[:], t3[:])
                return hp, sg

            def stage_a(n):
                """preproc + probe chain + mask multiply for image n."""
                o1, mt = o1s[n % NBUF], mts[n % NBUF]
                gs = gsp.tile([128, NQ], F32, name="gs")
                for q in range(NQ):
                    xin = xp.tile([128, H, W], F32, name="xin")
                    nc.sync.dma_start(xin[:], xa[n, 128 * q:128 * (q + 1), :, :])
                    nc.scalar.activation(
                        o1[:, q], xin[:], AF.Relu,
                        bias=sbn[q][:, 1:2], scale=sbn[q][:, 0:1],
                        accum_out=gs[:, q:q + 1],
                    )

                pre1a = pps.tile([128, 1], F32, name="pre1a", tag="pps")
                for q in range(NQ):
                    nc.tensor.matmul(pre1a[:], fc1wt[:, q, 0:128], gs[:, q:q + 1],
                                     start=(q == 0), stop=(q == NQ - 1))
                pre1b = pps.tile([HB, 1], F32, name="pre1b", tag="pps")
                for q in range(NQ):
                    nc.tensor.matmul(pre1b[:], fc1wt[:, q, 128:HID], gs[:, q:q + 1],
                                     start=(q == 0), stop=(q == NQ - 1))

                hp_a, sg_a = softplus_sigmoid(pre1a, fc1b5a[:, :], 128, "a")
                hp_b, sg_b = softplus_sigmoid(pre1b, fc1b5b, HB, "b")

                lg = pps.tile([1, NCLS], F32, name="lg", tag="pps")
                nc.tensor.matmul(lg[:], hp_a[:], fc2wt5a[:], start=True, stop=False)
                nc.tensor.matmul(lg[:], hp_b[:], fc2wt5b, start=False, stop=True)

                pslice = probe_acc[:, NCLS * n:NCLS * (n + 1)]
                nc.vector.tensor_add(pslice, lg[:], fc2brow)

                m1 = stile([1, 1], "m1")
                nc.vector.reduce_max(m1[:], pslice, axis=AX.X)
                eq1 = stile([1, NCLS], "eq1")
                nc.vector.tensor_scalar(eq1[:], pslice, m1[:], None, OP.is_equal)
                tmp = stile([1, NCLS], "tmp")
                nc.vector.scalar_tensor_tensor(tmp[:], eq1[:], -1e30, pslice,
                                               op0=OP.mult, op1=OP.add)
                m2 = stile([1, 1], "m2")
                nc.vector.reduce_max(m2[:], tmp[:], axis=AX.X)
                eq2 = stile([1, NCLS], "eq2")
                nc.vector.tensor_scalar(eq2[:], tmp[:], m2[:], None, OP.is_equal)
                e_row = stile([1, NCLS], "e_row")
                nc.vector.tensor_add(e_row[:], eq1[:], eq2[:])

                eT = pps.tile([NCLS, 1], F32, name="eT", tag="pps")
                nc.tensor.matmul(eT[:], e_row[:], ones_row[:, 0:1], start=True, stop=True)
                eT_sb = stile([NCLS, 1], "eT_sb")
                nc.vector.tensor_copy(eT_sb[:], eT[:])

                dh_a = pps.tile([128, 1], F32, name="dh_a", tag="pps")
                nc.tensor.matmul(dh_a[:], fc2w[:, 0:128], eT_sb[:], start=True, stop=True)
                dh_b = pps.tile([HB, 1], F32, name="dh_b", tag="pps")
                nc.tensor.matmul(dh_b[:], fc2w[:, 128:HID], eT_sb[:], start=True, stop=True)

                dpre_a = stile([128, 1], "dpre_a")
                nc.vector.tensor_mul(dpre_a[:], dh_a[:], sg_a[:])
                dpre_b = stile([HB, 1], "dpre_b")
                nc.vector.tensor_mul(dpre_b[:], dh_b[:], sg_b[:])

                exps = exp_p.tile([128, NQ], F32, name="exps")
                for q in range(NQ):
                    s_ps = pps.tile([128, 1], F32, name="s_ps", tag="pps")
                    nc.tensor.matmul(s_ps[:], fc1wa[:, 128 * q:128 * (q + 1)],
                                     dpre_a[:], start=True, stop=False)
                    nc.tensor.matmul(s_ps[:], fc1wb[:, 128 * q:128 * (q + 1)],
                                     dpre_b[:], start=False, stop=True)
                    nc.scalar.activation(exps[:, q:q + 1], s_ps[:], AF.Exp)

                zp = pps.tile([1, 1], F32, name="zp", tag="pps")
                for q in range(NQ):
                    nc.tensor.matmul(zp[:], exps[:, q:q + 1], ones_col[:],
                                     start=(q == 0), stop=(q == NQ - 1))
                r_sb = stile([1, 1], "r_sb")
                nc.vector.reciprocal(r_sb[:], zp[:])
                bc = pps.tile([128, 1], F32, name="bc", tag="pps")
                nc.tensor.matmul(bc[:], ones_row, r_sb[:], start=True, stop=True)

                # mask columns scaled by SA (fp8 activation scale)
                mcol = mcp.tile([128, NQ], F32, name="mcol")
                for q in range(NQ):
                    nc.vector.scalar_tensor_tensor(mcol[:, q:q + 1], exps[:, q:q + 1],
                                                   2.0 * SA, bc[:],
                                                   op0=OP.mult, op1=OP.mult)
                # masked_fp8 = out1_bf16 * (mask*SA) into padded interior
                for q in range(NQ):
                    nc.vector.tensor_scalar(mt[:, q, 1:33, 1:33], o1[:, q],
                                            mcol[:, q:q + 1], None, OP.mult)

            def stage_b1(n):
                """conv1 + bn2/relu for image n (fp8 DR)."""
                mt, ht = mts[n % NBUF], hts[n % NBUF]
                for co in range(NQ):
                    for hh in range(2):
                        ps = cps.tile([128, 16, 32], F32, name="ps1", tag="cps")
                        k = 0
                        for tap in range(9):
                            dy, dx = tap // 3, tap % 3
                            for pr in range(2):
                                nc.tensor.matmul(
                                    ps[:],
                                    w1t[:, tap, pr, :, 128 * co:128 * (co + 1)],
                                    mt[:, 2 * pr:2 * pr + 2,
                                       dy + 16 * hh:dy + 16 * hh + 16, dx:dx + 32],
                                    start=(k == 0), stop=(k == 17), perf_mode=DR,
                                )
                                k += 1
                        nc.scalar.activation(
                            ht[:, co, 1 + 16 * hh:17 + 16 * hh, 1:33], ps[:],
                            AF.Relu, bias=sbn[co][:, 3:4], scale=sbn[co][:, 2:3],
                        )

            def stage_b2(n):
                """conv2 + residual + store for image n (fp8 DR)."""
                ht = hts[n % NBUF]
                for co in range(NQ):
                    for hh in range(2):
                        ps = cps.tile([128, 16, 32], F32, name="ps2", tag="cps")
                        k = 0
                        for tap in range(9):
                            dy, dx = tap // 3, tap % 3
                            for pr in range(2):
                                nc.tensor.matmul(
                                    ps[:],
                                    w2t[:, tap, pr, :, 128 * co:128 * (co + 1)],
                                    ht[:, 2 * pr:2 * pr + 2,
                                       dy + 16 * hh:dy + 16 * hh + 16, dx:dx + 32],
                                    start=(k == 0), stop=(k == 17), perf_mode=DR,
                                )
                                k += 1
                        xres = xr.tile([128, 16, 32], F32, name="xres")
                        nc.sync.dma_start(
                            xres[:],
                            xa[n, 128 * co:128 * (co + 1), 16 * hh:16 * hh + 16, :])
                        o_sb = ost.tile([128, 16, 32], F32, name="o_sb")
                        nc.vector.scalar_tensor_tensor(o_sb[:], ps[:], 1.0 / (SW * SA),
                                                       xres[:], op0=OP.mult, op1=OP.add)
                        nc.sync.dma_start(
                            oa[n, 128 * co:128 * (co + 1), 16 * hh:16 * hh + 16, :],
                            o_sb[:])

            # software pipeline, conv2 staggered one stage behind conv1 so
            # every stage's dependencies are long since complete:
            #   a(0) a(1) b1(0) [a(2) b1(1) b2(0)] [a(3) b1(2) b2(1)] ...
            stage_a(0)
            if n_img > 1:
                stage_a(1)
            stage_b1(0)
            for n in range(n_img):
                if n + 2 < n_img:
                    stage_a(n + 2)
                if n + 1 < n_img:
                    stage_b1(n + 1)
                stage_b2(n)

            nc.sync.dma_start(pd.ap()[:], probe_acc)

    nc.compile()
    return nc


def prep_inputs(x, bn1_g, bn1_b, bn1_m, bn1_v, conv1_w,
                bn2_g, bn2_b, bn2_m, bn2_v, conv2_w,
                fc1_w, fc1_b, fc2_w, fc2_b, n_img=NIMG):
    """Host-side parameter prep -> list of per-core input maps."""
    eps = np.float32(1e-5)
    s1 = (bn1_g / np.sqrt(bn1_v + eps)).astype(np.float32)
    b1 = (bn1_b - bn1_m * s1).astype(np.float32)
    s2 = (bn2_g / np.sqrt(bn2_v + eps)).astype(np.float32)
    b2 = (bn2_b - bn2_m * s2).astype(np.float32)

    def wprep(w):
        # (Cout, Cin, 3, 3)*SW -> [tap, ci_in, pair, slab, co] fp8 e4m3
        wt = (w * SW).transpose(2, 3, 1, 0).reshape(9, 2, 2, 128, C)
        wt = wt.transpose(0, 3, 1, 2, 4)  # (9, 128, 2, 2, C)
        return np.ascontiguousarray(wt).astype(ml_dtypes.float8_e4m3)

    common = {
        "w1t": wprep(conv1_w),
        "w2t": wprep(conv2_w),
        # columns: s1, b1, s2/SW (psum scale), SA*b2 (fp8 out scale)
        "bnp": np.ascontiguousarray(
            np.stack([s1, b1, s2 / SW, SA * b2], axis=1)).astype(np.float32),
        "fc1wt": np.ascontiguousarray(
            (fc1_w.T / 1024.0).astype(np.float32)
            .reshape(NQ, 128, HID).transpose(1, 0, 2)),
        "fc1wa": np.ascontiguousarray(fc1_w[0:128]).astype(np.float32),
        "fc1wb": np.ascontiguousarray(fc1_w[128:HID]).astype(np.float32),
        "fc2wt5a": np.ascontiguousarray((fc2_w.T / 5.0)[0:128]).astype(np.float32),
        "fc2wt5b": np.ascontiguousarray((fc2_w.T / 5.0)[128:HID]).astype(np.float32),
        "fc2w": np.ascontiguousarray(fc2_w).astype(np.float32),
        "fc1b5a": (5.0 * fc1_b)[0:128].reshape(128, 1).astype(np.float32),
        "fc1b5b": (5.0 * fc1_b)[128:HID].reshape(HB, 1).astype(np.float32),
        "fc2brow": fc2_b.reshape(1, NCLS).astype(np.float32),
    }
    n_cores = x.shape[0] // n_img
    maps = []
    for i in range(n_cores):
        m = dict(common)
        m["x"] = np.ascontiguousarray(x[i * n_img:(i + 1) * n_img]).astype(np.float32)
        maps.append(m)
    return maps


def kernel(**inputs):
    inputs = {k: np.asarray(v) for k, v in inputs.items()}
    if "nc" not in _cache:
        _cache["nc"] = build_program()
    nc = _cache["nc"]
    in_maps = prep_inputs(**inputs)
    trace = bool(int(os.environ.get("KBENCH_TRACE", "0")))
    res = run_bass_kernel_spmd(nc, in_maps, list(range(NCORES)), trace=trace)
    _cache["last_results"] = res
    out = np.concatenate([res.results[i]["out"] for i in range(NCORES)], axis=0)
    probe = np.concatenate(
        [res.results[i]["probe"].reshape(NIMG, NCLS) for i in range(NCORES)], axis=0)
    return (out, probe)


# revision 16
# speedup vs baseline: 1.0393x; 1.0328x over previous
"""Trainium2 Bass kernel for nn_BasicBlock_CSAFR (topk_masking).

Computation per sample n (fully sample-independent -> pure data parallel,
8 images per NeuronCore across 8 cores):

  out1   = relu(bn1(x))                                  (512,32,32)
  gsum   = out1.sum(H,W)                                 (512,)
  z      = 5*(fc1_w @ (gsum/1024) + fc1_b)               (240,)
  hp     = softplus(z);  sig = sigmoid(z)
  logits = fc2_w @ (hp/5) + fc2_b                        (10,)   -> pred_probe
  e      = onehot(top1)+onehot(top2) of logits
  dh     = fc2_w.T @ e ; dpre = dh*sig ; s = fc1_w.T @ dpre
  mask   = 2*softmax(s)                                  (512,)
  h      = conv2(relu(bn2(conv1(out1*mask))))            3x3 SAME convs, 512->512
  out    = x + h

Convs run on the TensorEngine as 9 shifted matmuls per output tile
(channels on partitions, zero-padded 34x34 images in SBUF) in fp8-e4m3
DoubleRow mode (2 MACs/cell/cycle, 256-channel contraction per matmul)
with fp32 PSUM accumulation.  Activations/weights are pre-scaled
(SA=1024, SW=64) to sit in the fp8 normal range; scales fold into the
bn eviction and the residual add.  The final out = x + h is dominated
by exact fp32 x, so fp8 conv error (~1e-3 absmax) is far inside any
reasonable tolerance.  The probe/mask chain runs in fp32 using small
matmuls (incl. matmul-based transposes / partition broadcasts /
partition reductions).  softplus/sigmoid are built from the Exp LUT +
a deg-8 log1p polynomial on DVE (no Softplus table exists on trn2).
"""

import os
import sys

for _p in ("/opt/trn_rl_repo", "/root/.axon_site/_ro/trn_rl_repo"):
    if os.path.isdir(_p) and _p not in sys.path:
        sys.path.append(_p)

import numpy as np
import ml_dtypes

import concourse.bass as bass
import concourse.tile as tile
from concourse import bacc, mybir
from concourse.bass_utils import run_bass_kernel_spmd

F32 = mybir.dt.float32
BF16 = mybir.dt.bfloat16
FP8 = mybir.dt.float8e4
AF = mybir.ActivationFunctionType
OP = mybir.AluOpType
AX = mybir.AxisListType
DR = mybir.MatmulPerfMode.DoubleRow

N, C, H, W = 64, 512, 32, 32
HID, NCLS = 240, 10
NCORES = 8
NIMG = N // NCORES  # images per core
NQ = C // 128       # channel chunks
HB = HID - 128      # second fc-hidden chunk (112)
SW = 64.0           # fp8 weight scale
SA = 1024.0         # fp8 activation scale
NBUF = 3            # image pipeline depth

# log1p(u) ~= sum_k c_k u^k on [0,1]; max abs err 1.4e-7
LOG1P_C = [0.9999998102, -0.4999744938, 0.3327617657, -0.2449961172,
           0.1775702399, -0.1078536792, 0.04421419234, -0.008574676205]

_cache = {}


def build_program(n_img=NIMG):
    """Build + compile the per-core Bass program (n_img images)."""
    nc = bacc.Bacc("TRN2", target_bir_lowering=False, debug=False)

    xd = nc.dram_tensor("x", [n_img, C, H, W], F32, kind="ExternalInput")
    w1d = nc.dram_tensor("w1t", [9, 128, 2, 2, C], FP8, kind="ExternalInput")
    w2d = nc.dram_tensor("w2t", [9, 128, 2, 2, C], FP8, kind="ExternalInput")
    bnd = nc.dram_tensor("bnp", [C, 4], F32, kind="ExternalInput")
    f1td = nc.dram_tensor("fc1wt", [128, NQ, HID], F32, kind="ExternalInput")
    f1ad = nc.dram_tensor("fc1wa", [128, C], F32, kind="ExternalInput")
    f1bd = nc.dram_tensor("fc1wb", [HB, C], F32, kind="ExternalInput")
    f2tad = nc.dram_tensor("fc2wt5a", [128, NCLS], F32, kind="ExternalInput")
    f2tbd = nc.dram_tensor("fc2wt5b", [HB, NCLS], F32, kind="ExternalInput")
    f2d = nc.dram_tensor("fc2w", [NCLS, HID], F32, kind="ExternalInput")
    b1ad = nc.dram_tensor("fc1b5a", [128, 1], F32, kind="ExternalInput")
    b1bd = nc.dram_tensor("fc1b5b", [HB, 1], F32, kind="ExternalInput")
    b2d = nc.dram_tensor("fc2brow", [1, NCLS], F32, kind="ExternalInput")

    od = nc.dram_tensor("out", [n_img, C, H, W], F32, kind="ExternalOutput")
    pd = nc.dram_tensor("probe", [1, n_img * NCLS], F32, kind="ExternalOutput")

    xa, oa = xd.ap(), od.ap()

    with tile.TileContext(nc) as tc, tc.tile_pool(name="pp", bufs=1) as pp:
        def ptile(shape, dtype, name):
            return pp.tile(shape, dtype, name=name, tag=name)

        # ---- persistent tiles -------------------------------------------
        w1t = ptile([128, 9, 2, 2, C], FP8, "w1t_s")
        w2t = ptile([128, 9, 2, 2, C], FP8, "w2t_s")

        sbn = []
        for q in range(NQ):
            t = ptile([128, 4], F32, f"sbn{q}")
            nc.sync.dma_start(t[:], bnd.ap()[128 * q:128 * (q + 1), :])
            sbn.append(t)

        fc1wt = ptile([128, NQ, HID], F32, "fc1wt_s")
        nc.sync.dma_start(fc1wt[:], f1td.ap()[:])
        fc1wa = ptile([128, C], F32, "fc1wa_s")
        nc.sync.dma_start(fc1wa[:], f1ad.ap()[:])
        fc1wb_f = ptile([128, C], F32, "fc1wb_s")
        fc1wb = fc1wb_f[0:HB, :]
        nc.sync.dma_start(fc1wb, f1bd.ap()[:])
        fc2wt5a = ptile([128, NCLS], F32, "fc2wt5a_s")
        nc.sync.dma_start(fc2wt5a[:], f2tad.ap()[:])
        fc2wt5b_f = ptile([128, NCLS], F32, "fc2wt5b_s")
        fc2wt5b = fc2wt5b_f[0:HB, :]
        nc.sync.dma_start(fc2wt5b, f2tbd.ap()[:])
        fc2w_f = ptile([128, HID], F32, "fc2w_s")
        fc2w = fc2w_f[0:NCLS, :]
        nc.sync.dma_start(fc2w, f2d.ap()[:])
        fc1b5a = ptile([128, 1], F32, "fc1b5a_s")
        nc.sync.dma_start(fc1b5a[:], b1ad.ap()[:])
        fc1b5b_f = ptile([128, 1], F32, "fc1b5b_s")
        fc1b5b = fc1b5b_f[0:HB, :]
        nc.sync.dma_start(fc1b5b, b1bd.ap()[:])
        fc2brow_f = ptile([128, NCLS], F32, "fc2brow_s")
        fc2brow = fc2brow_f[0:1, :]
        nc.sync.dma_start(fc2brow, b2d.ap()[:])

        ones_row_f = ptile([128, 128], F32, "ones_row")
        ones_row = ones_row_f[0:1, :]
        nc.vector.memset(ones_row, 1.0)
        ones_col = ptile([128, 1], F32, "ones_col")
        nc.vector.memset(ones_col[:], 1.0)

        probe_acc_f = ptile([128, n_img * NCLS], F32, "probe_acc")
        probe_acc = probe_acc_f[0:1, :]

        # activation tiles (rotating over images); fp8 pads stay 0
        o1s = [ptile([128, NQ, H, W], BF16, f"o1_{i}") for i in range(NBUF)]
        mts = [ptile([128, NQ, 34, 34], FP8, f"mt{i}") for i in range(NBUF)]
        hts = [ptile([128, NQ, 34, 34], FP8, f"ht{i}") for i in range(NBUF)]
        for i in range(NBUF):
            nc.gpsimd.memset(mts[i][:], 0.0)
            nc.gpsimd.memset(hts[i][:], 0.0)

        with (
            tc.tile_pool(name="xp", bufs=6) as xp,
            tc.tile_pool(name="xr", bufs=4) as xr,
            tc.tile_pool(name="ost", bufs=4) as ost,
            tc.tile_pool(name="gsp", bufs=2) as gsp,
            tc.tile_pool(name="exp_p", bufs=2) as exp_p,
            tc.tile_pool(name="mcp", bufs=2) as mcp,
            tc.tile_pool(name="smp", bufs=2) as smp,
            tc.tile_pool(name="cps", bufs=3, space="PSUM") as cps,
            tc.tile_pool(name="pps", bufs=4, space="PSUM") as pps,
        ):
            def stile(shape, name):
                return smp.tile(shape, F32, name=name, tag=name)

            def softplus_sigmoid(pre_ps, bias_ap, P, sfx):
                """hp = softplus(5*pre+bias5), sig = sigmoid(same), on P rows."""
                z = stile([P, 1], f"z_{sfx}")
                nc.vector.tensor_scalar(z[:], pre_ps[:], 5.0, bias_ap, OP.mult, OP.add)
                nz = stile([P, 1], f"nz_{sfx}")
                nc.vector.tensor_scalar(nz[:], z[:], -1.0, None, OP.mult)
                mz = stile([P, 1], f"mz_{sfx}")
                nc.vector.tensor_tensor(mz[:], z[:], nz[:], op=OP.min)
                u = stile([P, 1], f"u_{sfx}")
                nc.scalar.activation(u[:], mz[:], AF.Exp)
                q = stile([P, 1], f"q_{sfx}")
                nc.vector.tensor_scalar(q[:], u[:], LOG1P_C[-1], None, OP.mult)
                for ck in reversed(LOG1P_C[:-1]):
                    nc.vector.scalar_tensor_tensor(q[:], q[:], float(ck), u[:],
                                                   op0=OP.add, op1=OP.mult)
                rz = stile([P, 1], f"rz_{sfx}")
                nc.vector.tensor_scalar(rz[:], z[:], 0.0, None, OP.max)
                hp = stile([P, 1], f"hp_{sfx}")
                nc.vector.tensor_add(hp[:], rz[:], q[:])
                # sigmoid: r = 1/(1+u); sig = (1-r) + m*(2r-1), m = [z>=0]
                m = stile([P, 1], f"m_{sfx}")
                nc.vector.tensor_scalar(m[:], z[:], 0.0, None, OP.is_ge)
                t1 = stile([P, 1], f"t1_{sfx}")
                nc.vector.tensor_scalar(t1[:], u[:], 1.0, None, OP.add)
                r = stile([P, 1], f"r_{sfx}")
                nc.vector.reciprocal(r[:], t1[:])
                t2 = stile([P, 1], f"t2_{sfx}")
                nc.vector.tensor_scalar(t2[:], r[:], 2.0, -1.0, OP.mult, OP.add)
                t3 = stile([P, 1], f"t3_{sfx}")
                nc.vector.tensor_scalar(t3[:], r[:], -1.0, 1.0, OP.mult, OP.add)
                t4 = stile([P, 1], f"t4_{sfx}")
                nc.vector.tensor_mul(t4[:], m[:], t2[:])
                sg = stile([P, 1], f"sg_{sfx}")
                nc.vector.tensor_add(sg[:], t4[:], t3[:])
                return hp, sg

            def stage_a(n):
                """preproc + probe chain + mask multiply for image n."""
                o1, mt = o1s[n % NBUF], mts[n % NBUF]
                gs = gsp.tile([128, NQ], F32, name="gs")
                for q in range(NQ):
                    xin = xp.tile([128, H, W], F32, name="xin")
                    nc.sync.dma_start(xin[:], xa[n, 128 * q:128 * (q + 1), :, :])
                    nc.scalar.activation(
                        o1[:, q], xin[:], AF.Relu,
                        bias=sbn[q][:, 1:2], scale=sbn[q][:, 0:1],
                        accum_out=gs[:, q:q + 1],
                    )

                pre1a = pps.tile([128, 1], F32, name="pre1a", tag="pps")
                for q in range(NQ):
                    nc.tensor.matmul(pre1a[:], fc1wt[:, q, 0:128], gs[:, q:q + 1],
                                     start=(q == 0), stop=(q == NQ - 1))
                pre1b = pps.tile([HB, 1], F32, name="pre1b", tag="pps")
                for q in range(NQ):
                    nc.tensor.matmul(pre1b[:], fc1wt[:, q, 128:HID], gs[:, q:q + 1],
                                     start=(q == 0), stop=(q == NQ - 1))

                hp_a, sg_a = softplus_sigmoid(pre1a, fc1b5a[:, :], 128, "a")
                hp_b, sg_b = softplus_sigmoid(pre1b, fc1b5b, HB, "b")

                lg = pps.tile([1, NCLS], F32, name="lg", tag="pps")
                nc.tensor.matmul(lg[:], hp_a[:], fc2wt5a[:], start=True, stop=False)
                nc.tensor.matmul(lg[:], hp_b[:], fc2wt5b, start=False, stop=True)

                pslice = probe_acc[:, NCLS * n:NCLS * (n + 1)]
                nc.vector.tensor_add(pslice, lg[:], fc2brow)

                m1 = stile([1, 1], "m1")
                nc.vector.reduce_max(m1[:], pslice, axis=AX.X)
                eq1 = stile([1, NCLS], "eq1")
                nc.vector.tensor_scalar(eq1[:], pslice, m1[:], None, OP.is_equal)
                tmp = stile([1, NCLS], "tmp")
                nc.vector.scalar_tensor_tensor(tmp[:], eq1[:], -1e30, pslice,
                                               op0=OP.mult, op1=OP.add)
                m2 = stile([1, 1], "m2")
                nc.vector.reduce_max(m2[:], tmp[:], axis=AX.X)
                eq2 = stile([1, NCLS], "eq2")
                nc.vector.tensor_scalar(eq2[:], tmp[:], m2[:], None, OP.is_equal)
                e_row = stile([1, NCLS], "e_row")
                nc.vector.tensor_add(e_row[:], eq1[:], eq2[:])

                eT = pps.tile([NCLS, 1], F32, name="eT", tag="pps")
                nc.tensor.matmul(eT[:], e_row[:], ones_row[:, 0:1], start=True, stop=True)
                eT_sb = stile([NCLS, 1], "eT_sb")
                nc.vector.tensor_copy(eT_sb[:], eT[:])

                dh_a = pps.tile([128, 1], F32, name="dh_a", tag="pps")
                nc.tensor.matmul(dh_a[:], fc2w[:, 0:128], eT_sb[:], start=True, stop=True)
                dh_b = pps.tile([HB, 1], F32, name="dh_b", tag="pps")
                nc.tensor.matmul(dh_b[:], fc2w[:, 128:HID], eT_sb[:], start=True, stop=True)

                dpre_a = stile([128, 1], "dpre_a")
                nc.vector.tensor_mul(dpre_a[:], dh_a[:], sg_a[:])
                dpre_b = stile([HB, 1], "dpre_b")
                nc.vector.tensor_mul(dpre_b[:], dh_b[:], sg_b[:])

                exps = exp_p.tile([128, NQ], F32, name="exps")
                for q in range(NQ):
                    s_ps = pps.tile([128, 1], F32, name="s_ps", tag="pps")
                    nc.tensor.matmul(s_ps[:], fc1wa[:, 128 * q:128 * (q + 1)],
                                     dpre_a[:], start=True, stop=False)
                    nc.tensor.matmul(s_ps[:], fc1wb[:, 128 * q:128 * (q + 1)],
                                     dpre_b[:], start=False, stop=True)
                    nc.scalar.activation(exps[:, q:q + 1], s_ps[:], AF.Exp)

                zp = pps.tile([1, 1], F32, name="zp", tag="pps")
                for q in range(NQ):
                    nc.tensor.matmul(zp[:], exps[:, q:q + 1], ones_col[:],
                                     start=(q == 0), stop=(q == NQ - 1))
                r_sb = stile([1, 1], "r_sb")
                nc.vector.reciprocal(r_sb[:], zp[:])
                bc = pps.tile([128, 1], F32, name="bc", tag="pps")
                nc.tensor.matmul(bc[:], ones_row, r_sb[:], start=True, stop=True)

                # mask columns scaled by SA (fp8 activation scale)
                mcol = mcp.tile([128, NQ], F32, name="mcol")
                for q in range(NQ):
                    nc.vector.scalar_tensor_tensor(mcol[:, q:q + 1], exps[:, q:q + 1],
                                                   2.0 * SA, bc[:],
                                                   op0=OP.mult, op1=OP.mult)
                # masked_fp8 = out1_bf16 * (mask*SA) into padded interior
                for q in range(NQ):
                    nc.vector.tensor_scalar(mt[:, q, 1:33, 1:33], o1[:, q],
                                            mcol[:, q:q + 1], None, OP.mult)

            def stage_b1(n):
                """conv1 + bn2/relu for image n (fp8 DR)."""
                mt, ht = mts[n % NBUF], hts[n % NBUF]
                for co in range(NQ):
                    for hh in range(2):
                        ps = cps.tile([128, 16, 32], F32, name="ps1", tag="cps")
                        k = 0
                        for tap in range(9):
                            dy, dx = tap // 3, tap % 3
                            for pr in range(2):
                                nc.tensor.matmul(
                                    ps[:],
                                    w1t[:, tap, pr, :, 128 * co:128 * (co + 1)],
                                    mt[:, 2 * pr:2 * pr + 2,
                                       dy + 16 * hh:dy + 16 * hh + 16, dx:dx + 32],
                                    start=(k == 0), stop=(k == 17), perf_mode=DR,
                                )
                                k += 1
                        nc.scalar.activation(
                            ht[:, co, 1 + 16 * hh:17 + 16 * hh, 1:33], ps[:],
                            AF.Relu, bias=sbn[co][:, 3:4], scale=sbn[co][:, 2:3],
                        )

            def stage_b2(n):
                """conv2 + residual + store for image n (fp8 DR)."""
                ht = hts[n % NBUF]
                for co in range(NQ):
                    for hh in range(2):
                        ps = cps.tile([128, 16, 32], F32, name="ps2", tag="cps")
                        k = 0
                        for tap in range(9):
                            dy, dx = tap // 3, tap % 3
                            for pr in range(2):
                                nc.tensor.matmul(
                                    ps[:],
                                    w2t[:, tap, pr, :, 128 * co:128 * (co + 1)],
                                    ht[:, 2 * pr:2 * pr + 2,
                                       dy + 16 * hh:dy + 16 * hh + 16, dx:dx + 32],
                                    start=(k == 0), stop=(k == 17), perf_mode=DR,
                                )
                                k += 1
                        xres = xr.tile([128, 16, 32], F32, name="xres")
                        nc.sync.dma_start(
                            xres[:],
                            xa[n, 128 * co:128 * (co + 1), 16 * hh:16 * hh + 16, :])
                        o_sb = ost.tile([128, 16, 32], F32, name="o_sb")
                        nc.vector.scalar_tensor_tensor(o_sb[:], ps[:], 1.0 / (SW * SA),
                                                       xres[:], op0=OP.mult, op1=OP.add)
                        nc.sync.dma_start(
                            oa[n, 128 * co:128 * (co + 1), 16 * hh:16 * hh + 16, :],
                            o_sb[:])

            # software pipeline, conv2 staggered one stage behind conv1 so
            # every stage's dependencies are long since complete:
            #   a(0) a(1) b1(0) [a(2) b1(1) b2(0)] [a(3) b1(2) b2(1)] ...
            stage_a(0)
            # conv1 weights stream after image-0 preproc loads; conv2
            # weights after image-1 preproc (needed a full stage later)
            for tap in range(9):
                nc.sync.dma_start(w1t[:, tap, :, :, :], w1d.ap()[tap])
            if n_img > 1:
                stage_a(1)
            for tap in range(9):
                nc.sync.dma_start(w2t[:, tap, :, :, :], w2d.ap()[tap])
            stage_b1(0)
            for n in range(n_img):
                if n + 2 < n_img:
                    stage_a(n + 2)
                if n + 1 < n_img:
                    stage_b1(n + 1)
                stage_b2(n)

            nc.sync.dma_start(pd.ap()[:], probe_acc)

    nc.compile()
    return nc


def prep_inputs(x, bn1_g, bn1_b, bn1_m, bn1_v, conv1_w,
                bn2_g, bn2_b, bn2_m, bn2_v, conv2_w,
                fc1_w, fc1_b, fc2_w, fc2_b, n_img=NIMG):
    """Host-side parameter prep -> list of per-core input maps."""
    eps = np.float32(1e-5)
    s1 = (bn1_g / np.sqrt(bn1_v + eps)).astype(np.float32)
    b1 = (bn1_b - bn1_m * s1).astype(np.float32)
    s2 = (bn2_g / np.sqrt(bn2_v + eps)).astype(np.float32)
    b2 = (bn2_b - bn2_m * s2).astype(np.float32)

    def wprep(w):
        # (Cout, Cin, 3, 3)*SW -> [tap, ci_in, pair, slab, co] fp8 e4m3
        wt = (w * SW).transpose(2, 3, 1, 0).reshape(9, 2, 2, 128, C)
        wt = wt.transpose(0, 3, 1, 2, 4)  # (9, 128, 2, 2, C)
        return np.ascontiguousarray(wt).astype(ml_dtypes.float8_e4m3)

    common = {
        "w1t": wprep(conv1_w),
        "w2t": wprep(conv2_w),
        # columns: s1, b1, s2/SW (psum scale), SA*b2 (fp8 out scale)
        "bnp": np.ascontiguousarray(
            np.stack([s1, b1, s2 / SW, SA * b2], axis=1)).astype(np.float32),
        "fc1wt": np.ascontiguousarray(
            (fc1_w.T / 1024.0).astype(np.float32)
            .reshape(NQ, 128, HID).transpose(1, 0, 2)),
        "fc1wa": np.ascontiguousarray(fc1_w[0:128]).astype(np.float32),
        "fc1wb": np.ascontiguousarray(fc1_w[128:HID]).astype(np.float32),
        "fc2wt5a": np.ascontiguousarray((fc2_w.T / 5.0)[0:128]).astype(np.float32),
        "fc2wt5b": np.ascontiguousarray((fc2_w.T / 5.0)[128:HID]).astype(np.float32),
        "fc2w": np.ascontiguousarray(fc2_w).astype(np.float32),
        "fc1b5a": (5.0 * fc1_b)[0:128].reshape(128, 1).astype(np.float32),
        "fc1b5b": (5.0 * fc1_b)[128:HID].reshape(HB, 1).astype(np.float32),
        "fc2brow": fc2_b.reshape(1, NCLS).astype(np.float32),
    }
    n_cores = x.shape[0] // n_img
    maps = []
    for i in range(n_cores):
        m = dict(common)
        m["x"] = np.ascontiguousarray(x[i * n_img:(i + 1) * n_img]).astype(np.float32)
        maps.append(m)
    return maps


def kernel(**inputs):
    inputs = {k: np.asarray(v) for k, v in inputs.items()}
    if "nc" not in _cache:
        _cache["nc"] = build_program()
    nc = _cache["nc"]
    in_maps = prep_inputs(**inputs)
    trace = bool(int(os.environ.get("KBENCH_TRACE", "0")))
    res = run_bass_kernel_spmd(nc, in_maps, list(range(NCORES)), trace=trace)
    _cache["last_results"] = res
    out = np.concatenate([res.results[i]["out"] for i in range(NCORES)], axis=0)
    probe = np.concatenate(
        [res.results[i]["probe"].reshape(NIMG, NCLS) for i in range(NCORES)], axis=0)
    return (out, probe)
